# revision 3
# baseline (speedup 1.0000x reference)
"""Trainium2 Bass kernel for GCN+RNN (nn_GCNN_RNN_32461362823865).

Strategy (v4):
  - Host: dense normalized adjacency A^T (fp16, 3072-padded), fold
    W2 = W @ W_ih.T and c0 = b @ W_ih.T + b_ih + b_hh, pre-transpose x.
  - Fused startup: DMA order x0-2, A blocks, x3-15 (one sync queue, so A
    streams at full BW right after the first 3 samples). U-block-0
    (dc0-6, 7 psum banks) chases A's arrival kb-by-kb; z for s0-2 rides
    along on the 8th bank. PE is busy from ~10us instead of idling
    through a 92us DMA window.
  - z lives in per-M-block rotating buffers zb[k] (bufs=3) instead of a
    flat 37.5KB tensor; z for s3-15 is woven into U-blocks 1-4 with a
    one-kb software pipeline (z(kb+1) emitted before U(kb)).
  - 7 a2a rounds, one triggered at each M-block end; RNN steps weave
    into blocks 2-6 at ~24 slots/block; tail = steps 112-127.
  - RNN: ring buffer [128 part, 10 slots x 384]: rows 0:50 = h (written
    by ScalarE tanh), rows 64:114 = U (DMA'd from a2a output, gpsimd
    queue gated on round triggers). One 128-contraction MM per step;
    tail steps split into column halves across two psum banks.

  Sample->core map: core c, round r holds global samples
  BOFF[r] + SR[r]*c + s4, SR = (2,3,2,3,2,3,1).
"""
import numpy as np

import concourse.bacc as bacc
import concourse.mybir as mybir
from concourse import tile
from concourse.bass_utils import run_bass_kernel_spmd

# ---- problem constants (hardcoded per contract) ----
N = 3070          # nodes
NP = 3072         # padded nodes (24 * 128, 8 * 384)
F = 128           # input features
J = 50            # folded feature dim (= RNN hidden)
B = 128           # batch (RNN sequence length)
NCORES = 8
S = B // NCORES   # samples per core = 16
NPC = NP // NCORES  # nodes per core = 384
KB = NP // 128    # 24 contraction blocks
SJ = S * J        # 800 U^T rows per core
NMB = 7           # M-blocks of 128 rows (last = 32)

SR = (2, 3, 2, 3, 2, 3, 1)       # samples per round per core
R = len(SR)
BOFF = [0, 16, 40, 56, 80, 96, 120]   # global step offset per round
ROFF = [0, 100, 250, 350, 500, 600, 750, 800]  # U^T row offset per round
RJ = [SR[r] * J for r in range(R)]
TRIG_BLK = {r: r for r in range(R)}  # round r triggers after block r
# U-phase dest-core passes: 5 + 3 psum banks (zb's z-psum holds 1,
# chase holds 7; post-chase: upsum 5 + rnnpp 2 + zp 1 = 8)
DCP = [list(range(5)), [5, 6, 7]]

# z sample groups: (block, pass) -> (s_lo, s_hi); chase covers s0-2.
ZW = {(1, 0): (3, 6), (2, 0): (6, 8), (2, 1): (8, 10),
      (3, 0): (10, 12), (3, 1): (12, 14), (4, 0): (14, 16)}

RING = 12         # rnn ring slots
TAIL_B = 112      # steps >= TAIL_B use the split (2 MM + 2 ACT) form
PRE = 8           # U prestage distance (steps ahead)

# weave schedule: M-block -> rnn steps woven into it
WEAVE = {2: list(range(0, 16)), 3: list(range(16, 40)),
         4: list(range(40, 64)), 5: list(range(64, 88)),
         6: list(range(88, 112))}

F16 = mybir.dt.float16
F32 = mybir.dt.float32
TANH = mybir.ActivationFunctionType.Tanh

_PROGRAM_CACHE = {}


def _step_rc(b):
    """global step -> (round, src core, sample-within-block)."""
    r = 0
    while r + 1 < R and b >= BOFF[r + 1]:
        r += 1
    q = b - BOFF[r]
    return r, q // SR[r], q % SR[r]


# writeback groups aligned so ring slots never wrap mid-group:
# out[b] lives in slot (b+1) % RING rows 0:50.
WB_GROUPS = [(0, 3)] + [(3 + 4 * i, 4) for i in range(31)] + [(127, 1)]
WB_AFTER = {b0 + g - 1: (b0, g) for b0, g in WB_GROUPS}


def _build_program():
    if "nc" in _PROGRAM_CACHE:
        return _PROGRAM_CACHE["nc"]
    nc = bacc.Bacc("TRN2", target_bir_lowering=False, debug=False,
                   num_devices=NCORES)

    xT = nc.dram_tensor("xT", [S, F, N], F16, kind="ExternalInput")
    at = nc.dram_tensor("at", [NP, NP], F16, kind="ExternalInput")
    w2 = nc.dram_tensor("w2", [F, J], F16, kind="ExternalInput")
    ws = nc.dram_tensor("ws", [128, J], F16, kind="ExternalInput")
    c0 = nc.dram_tensor("c0", [J, 1], F32, kind="ExternalInput")
    h0T = nc.dram_tensor("h0T", [J, NPC], F16, kind="ExternalInput")
    out = nc.dram_tensor("out", [B, J, NPC], F16, kind="ExternalOutput")

    with tile.TileContext(nc) as tc:
        with (
            tc.tile_pool(name="consts", bufs=1) as consts,
            tc.tile_pool(name="persist", bufs=1) as persist,
            tc.tile_pool(name="zbp", bufs=3) as zbp,
            tc.tile_pool(name="xin", bufs=5) as xin,
            tc.tile_pool(name="stg", bufs=2) as stg_pool,
            tc.tile_pool(name="dram", bufs=1, space="DRAM") as dram,
        ):
            w2_sb = consts.tile([F, J], F16, tag="w2_sb")
            ws_sb = consts.tile([128, J], F16, tag="ws_sb")
            c0_sb = consts.tile([J, 1], F32, tag="c0_sb")
            nc.scalar.dma_start(w2_sb[:], w2[:])
            nc.scalar.dma_start(ws_sb[:], ws[:])
            nc.scalar.dma_start(c0_sb[:], c0[:])

            at_sb = persist.tile([128, KB * NP], F16, tag="at_sb")
            ring = persist.tile([128, RING * NPC], F16, tag="ring")
            nc.vector.memset(ring[:], 0.0)
            nc.scalar.dma_start(ring[0:J, 0:NPC], h0T[:])  # h0 -> slot 0

            a2a_in = [dram.tile([NCORES * RJ[r], NPC], F16, name=f"a2ai_{r}")
                      for r in range(R)]
            a2a_out = [dram.tile([NCORES * RJ[r], NPC], F16, name=f"a2ao_{r}")
                       for r in range(R)]
            warm_in = dram.tile([NCORES, 8192], F16, name="warm_in")
            warm_out = dram.tile([NCORES, 8192], F16, name="warm_out")

            # dummy collective: absorbs barrier + mesh warmup during the
            # DMA-bound chase window
            nc.gpsimd.collective_compute(
                "AllToAll", mybir.AluOpType.bypass,
                replica_groups=[list(range(NCORES))],
                ins=[warm_in.opt()], outs=[warm_out.opt()])

            state = {}
            trig_done = [False] * R
            pending_pre = []
            xbigs = {}
            zbufs = {}

            def load_x(s):
                xb = xin.tile([F, NP], F16, tag="xbig", name=f"xbig_{s}")
                xbigs[s] = xb
                nc.sync.dma_start(xb[:, 0:N], xT[s])
                nc.vector.memset(xb[:, N:NP], 0.0)

            def zbuf(k):
                if k not in zbufs:
                    zbufs[k] = zbp.tile([128, KB * 128], F16, tag="zb",
                                        name=f"zb_{k}")
                return zbufs[k]

            def zbatch(zp_pool, s_lo, s_hi, kb):
                """z for samples [s_lo,s_hi) at contraction block kb,
                copied into the per-M-block z buffers."""
                ns = s_hi - s_lo
                zt = zp_pool.tile([128, 300], F32, tag="zp",
                                  name=f"zp_{s_lo}_{kb}")
                for si in range(ns):
                    nc.tensor.matmul(
                        zt[:, si * J:(si + 1) * J],
                        xbigs[s_lo + si][:, kb * 128:(kb + 1) * 128],
                        w2_sb[:], start=True, stop=True)
                lo, hi = s_lo * J, s_hi * J       # global row range
                k0, k1 = lo // 128, (hi - 1) // 128
                for k in range(k0, k1 + 1):
                    blo, bhi = max(lo, k * 128), min(hi, (k + 1) * 128)
                    nc.vector.tensor_copy(
                        zbuf(k)[:, kb * 128 + blo - k * 128:
                                kb * 128 + bhi - k * 128],
                        zt[:, blo - lo:bhi - lo])

            def _do_prestage(b):
                r, c, s4 = _step_rc(b)
                slot = b % RING
                row = c * RJ[r] + s4 * J
                nc.gpsimd.dma_start(
                    ring[64:64 + J, slot * NPC:(slot + 1) * NPC],
                    a2a_out[r][row:row + J, :])

            def prestage_u(b):
                if trig_done[_step_rc(b)[0]]:
                    _do_prestage(b)
                else:
                    pending_pre.append(b)

            def trig(r):
                nc.gpsimd.collective_compute(
                    "AllToAll", mybir.AluOpType.bypass,
                    replica_groups=[list(range(NCORES))],
                    ins=[a2a_in[r].opt()],
                    outs=[a2a_out[r].opt()])
                trig_done[r] = True
                ready = [b for b in pending_pre if trig_done[_step_rc(b)[0]]]
                for b in ready:
                    pending_pre.remove(b)
                    _do_prestage(b)

            def rnn_step(b):
                slot = b % RING
                nslot = (b + 1) % RING
                if b + PRE < B:
                    prestage_u(b + PRE)
                rhs = ring[:, slot * NPC:(slot + 1) * NPC]
                dst = ring[0:J, nslot * NPC:(nslot + 1) * NPC]
                if b < TAIL_B:
                    pp = state["pp"].tile([J, NPC], F32, tag="pp",
                                          name=f"pp_{b}")
                    nc.tensor.matmul(pp[:], ws_sb[:], rhs,
                                     start=True, stop=True)
                    nc.scalar.activation(dst, pp[:], TANH,
                                         bias=c0_sb[:, 0:1])
                else:
                    H = NPC // 2
                    for half in range(2):
                        pph = state["pp"].tile([J, NPC], F32, tag="pp",
                                               name=f"pp_{b}_{half}")
                        nc.tensor.matmul(pph[:, 0:H], ws_sb[:],
                                         rhs[:, half * H:(half + 1) * H],
                                         start=True, stop=True)
                        nc.scalar.activation(
                            dst[:, half * H:(half + 1) * H], pph[:, 0:H],
                            TANH, bias=c0_sb[:, 0:1])
                if b in WB_AFTER:
                    b0, g = WB_AFTER[b]
                    s0 = (b0 + 1) % RING
                    nc.sync.dma_start(
                        out[b0:b0 + g].rearrange("g j n -> j g n"),
                        ring[0:J, s0 * NPC:(s0 + g) * NPC].rearrange(
                            "j (g n) -> j g n", g=g))

            def stage(k, dcs, st):
                """a2a staging DMAs for M-block k, dest cores `dcs`
                (consecutive), source st [mrows, len(dcs)*NPC] fp16."""
                row0 = k * 128
                mrows = min(128, SJ - row0)
                nd = len(dcs)
                dc0 = dcs[0]
                for r in range(R):
                    lo = max(row0, ROFF[r])
                    hi = min(row0 + mrows, ROFF[r + 1])
                    if lo >= hi:
                        continue
                    nc.sync.dma_start(
                        a2a_in[r].rearrange("(dc rw) n -> rw dc n",
                                            dc=NCORES)[
                            lo - ROFF[r]:hi - ROFF[r], dc0:dc0 + nd, :],
                        st[lo - row0:hi - row0, :].rearrange(
                            "rw (dc n) -> rw dc n", dc=nd))

            def stage_psums(k, dcs, psums):
                """copy psums (f32) -> fp16 staging tiles -> a2a_in, in
                chunks of <=2 consecutive dest cores."""
                row0 = k * 128
                mrows = min(128, SJ - row0)
                i = 0
                while i < len(dcs):
                    chunk = dcs[i:i + 2]
                    st = stg_pool.tile([128, 2 * NPC], F16, tag="st",
                                       name=f"st_{k}_{chunk[0]}")
                    for di, dc in enumerate(chunk):
                        nc.vector.tensor_copy(
                            st[0:mrows, di * NPC:(di + 1) * NPC],
                            psums[dc][0:mrows, :])
                    stage(k, chunk, st[0:mrows, 0:len(chunk) * NPC])
                    i += 2

            # ---- psum pool stack: zp (outermost, lives through blk4),
            # then chase (closes after block 0), then upsum + rnn pp ----
            zp_ctx = tc.tile_pool(name="zpsum", bufs=1, space="PSUM")
            zp = zp_ctx.__enter__()

            # ================= fused startup: chase A's DMA ==============
            with nc.named_scope("startup"):
                # DMA order on the sync queue: x0-2, A blocks, x3-15.
                for s in range(3):
                    load_x(s)
                for kb in range(KB):
                    nc.sync.dma_start(
                        at_sb[:, kb * NP:(kb + 1) * NP],
                        at[kb * 128:(kb + 1) * 128, :])
                for s in range(3, S):
                    load_x(s)

                with tc.tile_pool(name="chase", bufs=7,
                                  space="PSUM") as chase:
                    cps = {dc: chase.tile([128, NPC], F32, tag="ch",
                                          name=f"ch_{dc}")
                           for dc in range(7)}
                    for kb in range(KB):
                        zbatch(zp, 0, 3, kb)   # z s0-2 -> zb0, zb1
                        for dc in range(7):
                            nc.tensor.matmul(
                                cps[dc][:],
                                zbuf(0)[:, kb * 128:kb * 128 + 128],
                                at_sb[:, kb * NP + dc * NPC:
                                      kb * NP + (dc + 1) * NPC],
                                start=(kb == 0), stop=(kb == KB - 1))
                    stage_psums(0, list(range(6)), cps)

                    # dc6 staged below together with dc7 (one chunk each)
                    st6 = stg_pool.tile([128, 2 * NPC], F16, tag="st",
                                        name="st_0_6")
                    nc.vector.tensor_copy(st6[:, 0:NPC], cps[6][:])
                    stage(0, [6], st6[:, 0:NPC])

            upsum_ctx = tc.tile_pool(name="upsum", bufs=5, space="PSUM")
            upsum = upsum_ctx.__enter__()
            pp_ctx = tc.tile_pool(name="p3psum", bufs=2, space="PSUM")
            state["pp"] = pp_ctx.__enter__()

            with nc.named_scope("ummphase"):
                # finish block 0: dc7 (full 24-kb pass, A resident now)
                up7 = upsum.tile([128, NPC], F32, tag="up", name="up_0_7")
                for kb in range(KB):
                    nc.tensor.matmul(
                        up7[:], zbuf(0)[:, kb * 128:kb * 128 + 128],
                        at_sb[:, kb * NP + 7 * NPC:kb * NP + 8 * NPC],
                        start=(kb == 0), stop=(kb == KB - 1))
                st7 = stg_pool.tile([128, 2 * NPC], F16, tag="st",
                                    name="st_0_7")
                nc.vector.tensor_copy(st7[:, 0:NPC], up7[:])
                stage(0, [7], st7[:, 0:NPC])
                trig(0)

                for b in range(PRE):
                    prestage_u(b)

                # ================= U-phase blocks 1-6 ===================
                for k in range(1, NMB):
                    row0 = k * 128
                    mrows = min(128, SJ - row0)
                    steps = list(WEAVE.get(k, []))
                    w1, wrest = steps[:12], steps[12:]
                    for pi, dcs in enumerate(DCP):
                        zg = ZW.get((k, pi))
                        if zg:
                            zbatch(zp, zg[0], zg[1], 0)
                        psums = {dc: upsum.tile([128, NPC], F32, tag="up",
                                                name=f"up_{k}_{dc}")
                                 for dc in dcs}
                        wq = w1 if pi == 0 else wrest
                        for kb in range(KB):
                            if zg and kb + 1 < KB:
                                zbatch(zp, zg[0], zg[1], kb + 1)
                            for dc in dcs:
                                nc.tensor.matmul(
                                    psums[dc][0:mrows, :],
                                    zbufs[k][:, kb * 128:kb * 128 + mrows],
                                    at_sb[:, kb * NP + dc * NPC:
                                          kb * NP + (dc + 1) * NPC],
                                    start=(kb == 0), stop=(kb == KB - 1))
                            if kb % 2 == 1 and wq:
                                rnn_step(wq.pop(0))
                        stage_psums(k, dcs, psums)
                        while wq:
                            rnn_step(wq.pop(0))
                    for r, blk in TRIG_BLK.items():
                        if blk == k:
                            trig(r)

            # ================= rnn tail ==================================
            with nc.named_scope("rnn"):
                for b in range(TAIL_B, B):
                    rnn_step(b)

            pp_ctx.__exit__(None, None, None)
            upsum_ctx.__exit__(None, None, None)
            zp_ctx.__exit__(None, None, None)

    nc.compile()
    _PROGRAM_CACHE["nc"] = nc
    return nc


def _host_prep(x_in, edge_index, edge_weight, W, b, W_ih, W_hh, b_ih, b_hh, h0):
    """Build per-core input maps (all numpy, no device work)."""
    edge_index = np.asarray(edge_index).astype(np.int64)
    # exact reference remap: rank among unique ids (size=N, fill=2**30)
    uniq = np.unique(edge_index)
    if uniq.size < N:
        uniq = np.concatenate([uniq, np.full(N - uniq.size, 2 ** 30, np.int64)])
    else:
        uniq = uniq[:N]
    ei = np.searchsorted(uniq, edge_index)
    src, dst = ei[0], ei[1]

    ew = np.asarray(edge_weight, np.float64)
    deg = np.zeros(N, np.float64)
    np.add.at(deg, dst, ew)
    deg += 1.0  # self loops (weight 1)
    dinv = np.where(deg > 0, 1.0 / np.sqrt(deg), 0.0)

    AT = np.zeros((NP, NP), np.float32)
    np.add.at(AT, (src, dst), (dinv[src] * ew * dinv[dst]).astype(np.float32))
    idx = np.arange(N)
    AT[idx, idx] += (dinv * dinv).astype(np.float32)
    AT16 = AT.astype(np.float16)

    W = np.asarray(W, np.float32)
    W_ih = np.asarray(W_ih, np.float32)
    W2 = (W.astype(np.float64) @ W_ih.T.astype(np.float64)).astype(np.float16)
    c0 = (np.asarray(b, np.float32) @ W_ih.T + np.asarray(b_ih, np.float32)
          + np.asarray(b_hh, np.float32)).astype(np.float32).reshape(J, 1)
    ws = np.zeros((128, J), np.float32)
    ws[0:J] = np.asarray(W_hh, np.float32).T
    ws[64:64 + J] = np.eye(J, dtype=np.float32)
    ws = ws.astype(np.float16)

    x_in = np.asarray(x_in, np.float32)
    h0 = np.asarray(h0, np.float32)
    h0p = np.zeros((NP, J), np.float16)
    h0p[:N] = h0.astype(np.float16)

    in_maps = []
    for c in range(NCORES):
        samples = [BOFF[r] + SR[r] * c + s4
                   for r in range(R) for s4 in range(SR[r])]
        xc = x_in[samples]                                # (S, N, F)
        xTc = np.ascontiguousarray(
            xc.transpose(0, 2, 1)).astype(np.float16)     # (S, F, N)
        h0Tc = np.ascontiguousarray(
            h0p[c * NPC:(c + 1) * NPC].T)                 # (J, NPC)
        in_maps.append({"xT": xTc, "at": AT16, "w2": W2, "ws": ws,
                        "c0": c0, "h0T": h0Tc})
    return in_maps


def _assemble(results):
    parts = []
    for c in range(NCORES):
        o = results[c]["out"]                 # (B, J, NPC) fp16
        parts.append(np.ascontiguousarray(o.transpose(0, 2, 1)))  # (B, NPC, J)
    full = np.concatenate(parts, axis=1)      # (B, NP, J)
    return full[:, :N, :].astype(np.float32)


def run_internal(inputs, trace=False, trace_cores=None):
    nc = _build_program()
    in_maps = _host_prep(**inputs)
    res = run_bass_kernel_spmd(nc, in_maps, list(range(NCORES)), trace=trace,
                               trace_cores=trace_cores)
    return _assemble(res.results), res


def kernel(**inputs) -> np.ndarray:
    out, _ = run_internal(inputs, trace=False)
    return out
